# revision 2
# baseline (speedup 1.0000x reference)
"""Attn_LSTM Trainium2 kernel — 8-core data-parallel Bass/Tile implementation.

Model (per reference): 1-layer LSTM encoder over L=96 steps, then T=24
attention-decoder steps. B=4096 sharded 512/core across 8 NeuronCores;
weights replicated.

Key device-side design points:
  * All recurrent state is kept transposed ([H, B]) so the PE consumes h
    directly as lhsT with no per-step transposes on the recurrent path.
  * Attention context uses a Horner-form affine scan (tensor_tensor_scan):
      S_l = r_l * S_{l-1} + enc_l,  r_l = e_{l-1}/e_l = exp(z_{l-1}-z_l)
    so softmax-weight-and-reduce is ONE DVE pass over enc. The z-differences
    come straight out of the PE by using column-differenced attention weights,
    and a -1e30 in the difference-bias column 0 makes exp()=0 there, which
    resets the scan at every (h-row, chunk) boundary for free. A ones-row
    appended to enc yields the softmax denominator from the same scan.
  * The local walrus build accepts at most ONE semaphore wait per
    instruction; legalize_waits() splits extra waits onto same-engine NoOps.
"""

import numpy as np

import concourse.bass as bass
import concourse.tile as tile
from concourse import mybir
from concourse.masks import make_identity
from concourse.bass_utils import run_bass_kernel_spmd

H = 64
C = 8
L = 96
T = 24
B = 4096
NCORES = 8
BS = B // NCORES          # 512 batch per core
NCH = BS // 128           # 4 partition chunks per core

F32 = mybir.dt.float32
AF = mybir.ActivationFunctionType
ALU = mybir.AluOpType

NEG_BIG = -1.0e30


def _legalize_waits(nc):
    """This walrus build rejects >1 sem wait per instruction; split extras
    onto same-engine NoOps placed immediately before."""
    cnt = 0
    for bb in nc.main_func.blocks:
        new = []
        for inst in bb.instructions:
            si = inst.sync_info
            if si is not None and len(si.on_wait) > 1:
                waits = list(si.on_wait)
                for w in waits[:-1]:
                    nop = mybir.InstNoOp(name=f"wsplit-{cnt}", ins=[], outs=[])
                    cnt += 1
                    nop.engine = inst.engine
                    nop.sync_info = mybir.SyncInfo(on_wait=[w], on_update=[])
                    new.append(nop)
                inst.sync_info = mybir.SyncInfo(
                    on_wait=[waits[-1]], on_update=list(si.on_update))
            new.append(inst)
        bb.instructions = new
    return cnt


def _tts_raw(nc, out, data0, data1, initial, op0, op1):
    """tensor_tensor_scan without the 2D-shape assert: the recurrence chains
    across free dims, which we exploit (r=0 at row starts resets the state)."""
    eng = nc.vector
    return eng.add_instruction(
        mybir.InstTensorScalarPtr(
            name=nc.get_next_instruction_name(),
            is_tensor_tensor_scan=True,
            is_scalar_tensor_tensor=True,
            op0=op0,
            op1=op1,
            ins=[
                eng.lower_ap(data0),
                eng.lower_ap_or_imm(initial),
                eng.lower_ap(data1),
            ],
            outs=[eng.lower_ap(out)],
        )
    )


_REPEAT = 1
_DECODER = True
_ENCODER = True


def _build_program():
    nc = bass.Bass("TRN2", target_bir_lowering=False, debug=False,
                   num_devices=NCORES)

    ext_in = {}

    def din(name, shape):
        ext_in[name] = shape
        return nc.dram_tensor(name, list(shape), F32, kind="ExternalInput").ap()

    xT = din("xT", (L, C, BS))                 # normalized, transposed input
    enc_wih = din("enc_wih", (C, 4, H))        # gate order (i, f, o, g)
    enc_whh = din("enc_whh", (H, 4, H))
    enc_b = din("enc_b", (H, 4))
    emb_wT = din("emb_wT", (C, H))
    emb_b = din("emb_b", (H, 1))
    attn_we = din("attn_we", (H, L))           # DIFFERENCED emb-part weights
    attn_wh = din("attn_wh", (H + 1, L))       # rows 0-63 diff Wh, row 64 diff bias (+ -1e30 col0)
    wie = din("wie", (H, 4, H))                # dec_Wih @ comb_W1, packed
    wic = din("wic", (H, 4, H))                # dec_Wih @ comb_W2, packed
    dec_whh = din("dec_whh", (H, 4, H))
    dec_b = din("dec_b", (H, 4))
    out_wT = din("out_wT", (H, C))
    out_b = din("out_b", (C, 1))
    emb_whT = din("emb_whT", (H, H))      # (emb_W @ out_W).T
    emb_bh = din("emb_bh", (H, 1))        # emb_W @ out_b + emb_b

    preds = nc.dram_tensor("preds", [T, C, BS], F32, kind="ExternalOutput").ap()

    with tile.TileContext(nc) as tc:
        with (
            tc.tile_pool(name="state", bufs=1) as st,
            tc.tile_pool(name="xin", bufs=4) as xin,
            tc.tile_pool(name="scan", bufs=2) as tmpp,
            tc.tile_pool(name="gps", bufs=1, space="PSUM") as gps,
            tc.tile_pool(name="tps", bufs=1, space="PSUM") as tps,
        ):
            # ---------- persistent tiles ----------
            ident = st.tile([128, 128], F32)
            make_identity(nc, ident[:])

            w_enc_wih = st.tile([C, 4, H], F32)
            w_enc_whh = st.tile([H, 4, H], F32)
            w_enc_b = st.tile([H, 4], F32)
            w_emb_wT = st.tile([C, H], F32)
            w_emb_b = st.tile([H, 1], F32)
            w_attn_we = st.tile([H, L], F32)
            w_attn_wh = st.tile([H + 1, L], F32)
            w_wie = st.tile([H, 4, H], F32)
            w_wic = st.tile([H, 4, H], F32)
            w_dec_whh = st.tile([H, 4, H], F32)
            w_dec_b = st.tile([H, 4], F32)
            w_out_wT = st.tile([H, C], F32)
            w_out_b = st.tile([C, 1], F32)
            w_emb_whT = st.tile([H, H], F32)
            w_emb_bh = st.tile([H, 1], F32)
            for tl, ap in ((w_enc_wih, enc_wih), (w_enc_whh, enc_whh),
                           (w_enc_b, enc_b), (w_emb_wT, emb_wT),
                           (w_emb_b, emb_b), (w_attn_we, attn_we),
                           (w_attn_wh, attn_wh), (w_wie, wie), (w_wic, wic),
                           (w_dec_whh, dec_whh), (w_dec_b, dec_b),
                           (w_out_wT, out_wT), (w_out_b, out_b),
                           (w_emb_whT, emb_whT), (w_emb_bh, emb_bh)):
                nc.gpsimd.dma_start(tl[:], ap[:])

            # recurrent state, transposed; h row 64 is a ones-row feeding the
            # (differenced) attention bias
            h_T = st.tile([H + 1, BS], F32)
            c_T = st.tile([H, BS], F32)
            nc.vector.memset(h_T[:], 0.0)
            nc.vector.memset(c_T[:], 0.0)
            nc.vector.memset(h_T[H : H + 1, :], 1.0)

            # encoder outputs: [b, chunk, h, l]
            enc_sb = st.tile([128, NCH, H, L], F32)

            # gate tiles (SBUF) + cell temps
            g_sb = [st.tile([H, BS], F32, tag=f"g{gi}", name=f"g{gi}") for gi in range(4)]
            t1 = st.tile([H, BS], F32)
            t2 = st.tile([H, BS], F32)
            tc_sb = st.tile([H, BS], F32)

            # decoder tiles
            emb_sb = st.tile([H, BS], F32)
            e_sb = st.tile([128, NCH, L], F32)
            d_sb = st.tile([128, NCH], F32)
            rec_sb = st.tile([128, NCH], F32)
            ctx_ch = st.tile([128, NCH, H], F32)
            ctx_sb = st.tile([H, BS], F32)
            inp_sb = st.tile([C, BS], F32)

            # PSUM
            gate_ps = [gps.tile([H, BS], F32, tag=f"gp{gi}", name=f"gp{gi}") for gi in range(4)]
            tp_ps_pool = tps  # [128, NCH, H] tiles for encoder transposes

            ACTF = (AF.Sigmoid, AF.Sigmoid, AF.Sigmoid, AF.Tanh)

            for _rep in range(_REPEAT):

                def lstm_cell(bias_tile):
                    """gates (psum) -> h_T/c_T update. Gate order (i, f, o, g);
                    emission order (g, i, f, o) so the DVE chain starts early."""
                    for gi in (3, 0, 1, 2):
                        nc.scalar.activation(g_sb[gi][:], gate_ps[gi][:], ACTF[gi],
                                             bias=bias_tile[:, gi : gi + 1])
                    nc.vector.tensor_mul(t1[:], g_sb[0][:], g_sb[3][:])   # i*tanh(g)
                    nc.vector.tensor_mul(t2[:], g_sb[1][:], c_T[:])       # f*c
                    nc.vector.tensor_add(c_T[:], t1[:], t2[:])
                    nc.scalar.activation(tc_sb[:], c_T[:], AF.Tanh)
                    nc.vector.tensor_mul(h_T[0:H, :], g_sb[2][:], tc_sb[:])

                # ------------------ encoder ------------------
                for l in range(L if _ENCODER else 0):
                    x_t = xin.tile([C, BS], F32, tag="x")
                    nc.sync.dma_start(x_t[:], xT[l])
                    for gi in range(4):
                        nc.tensor.matmul(gate_ps[gi][:], w_enc_wih[:, gi, :],
                                         x_t[:], start=True, stop=False)
                        nc.tensor.matmul(gate_ps[gi][:], w_enc_whh[:, gi, :],
                                         h_T[0:H, :], start=False, stop=True)
                    lstm_cell(w_enc_b)
                    # store h (transposed back to [b, h]) into enc_sb[:, :, l]
                    tp = tp_ps_pool.tile([128, NCH, H], F32, tag="tp")
                    for ci in range(NCH):
                        nc.tensor.transpose(tp[:, ci, :],
                                            h_T[0:H, 128 * ci : 128 * (ci + 1)],
                                            ident[0:H, 0:H])
                    nc.scalar.copy(enc_sb[:, :, :, l], tp[:])

                # ------------------ decoder ------------------
                # initial decoder input = last normalized x = xT[L-1]
                nc.sync.dma_start(inp_sb[:], xT[L - 1])

                for t in range(T if _DECODER else 0):
                    # embedding: from raw input at t=0, from h directly after
                    # (out_W folded into emb_W, keeping pred off the path)
                    emb_ps = tps.tile([H, BS], F32, tag="sm")
                    if t == 0:
                        nc.tensor.matmul(emb_ps[:], w_emb_wT[:], inp_sb[:],
                                         start=True, stop=True)
                        nc.scalar.activation(emb_sb[:], emb_ps[:], AF.Relu,
                                             bias=w_emb_b[:, 0:1])
                    else:
                        nc.tensor.matmul(emb_ps[:], w_emb_whT[:], h_T[0:H, :],
                                         start=True, stop=True)
                        nc.scalar.activation(emb_sb[:], emb_ps[:], AF.Relu,
                                             bias=w_emb_bh[:, 0:1])

                    # differenced attention scores -> exp -> scan ratios
                    zd_ps = tps.tile([128, NCH, L], F32, tag="zd")
                    for ci in range(NCH):
                        sl = slice(128 * ci, 128 * (ci + 1))
                        nc.tensor.matmul(zd_ps[:, ci, :], emb_sb[:, sl],
                                         w_attn_we[:], start=True, stop=False)
                        nc.tensor.matmul(zd_ps[:, ci, :], h_T[:, sl],
                                         w_attn_wh[:], start=False, stop=True)
                    for ci in range(NCH):
                        nc.scalar.activation(e_sb[:, ci, :], zd_ps[:, ci, :],
                                             AF.Exp,
                                             accum_out=d_sb[:, ci : ci + 1])
                    nc.vector.reciprocal(rec_sb[:], d_sb[:])

                    # softmax-weighted sum: tmp = enc * e (h-bcast), reduce l
                    ctxT_ps = tps.tile([H, BS], F32, tag="ctxT")
                    for ci in range(NCH):
                        tmp = tmpp.tile([128, H, L], F32, tag="tmp")
                        eb = e_sb[:, ci, :].unsqueeze(1).broadcast_to((128, H, L))
                        nc.vector.tensor_mul(tmp[:], enc_sb[:, ci], eb)
                        craw = tmpp.tile([128, H], F32, tag="craw")
                        nc.vector.tensor_reduce(craw[:], tmp[:],
                                                axis=mybir.AxisListType.X,
                                                op=ALU.add)
                        nc.vector.tensor_scalar(
                            out=ctx_ch[:, ci, :], in0=craw[:],
                            scalar1=rec_sb[:, ci : ci + 1], scalar2=None,
                            op0=ALU.mult)
                        nc.tensor.transpose(ctxT_ps[:, 128 * ci : 128 * (ci + 1)],
                                            ctx_ch[:, ci, :], ident[:])
                    nc.scalar.copy(ctx_sb[:], ctxT_ps[:])

                    # decoder LSTM cell (comb layer folded into gate weights)
                    for gi in range(4):
                        nc.tensor.matmul(gate_ps[gi][:], w_wie[:, gi, :],
                                         emb_sb[:], start=True, stop=False)
                        nc.tensor.matmul(gate_ps[gi][:], w_dec_whh[:, gi, :],
                                         h_T[0:H, :], start=False, stop=False)
                        nc.tensor.matmul(gate_ps[gi][:], w_wic[:, gi, :],
                                         ctx_sb[:], start=False, stop=True)
                    lstm_cell(w_dec_b)

                    # prediction -> next input + output store
                    pred_ps = tps.tile([C, BS], F32, tag="sm")
                    nc.tensor.matmul(pred_ps[:], w_out_wT[:], h_T[0:H, :],
                                     start=True, stop=True)
                    nc.scalar.activation(inp_sb[:], pred_ps[:], AF.Identity,
                                         bias=w_out_b[:, 0:1])
                    nc.sync.dma_start(preds[t], inp_sb[:])

    _legalize_waits(nc)
    return nc


_NC_CACHE = []


def _get_nc():
    if not _NC_CACHE:
        _NC_CACHE.append(_build_program())
    return _NC_CACHE[0]


def _prep_weights(i):
    """Host-side packing. Gate order (i, f, o, g); PyTorch order is i,f,g,o."""
    idx = np.r_[0:64, 64:128, 192:256, 128:192]

    def pack(w):                       # [256, K] -> [K, 4, 64]
        return np.ascontiguousarray(
            w[idx].reshape(4, 64, -1).transpose(2, 0, 1).astype(np.float32))

    enc_wih = pack(i["enc_Wih"])
    enc_whh = pack(i["enc_Whh"])
    enc_b = np.ascontiguousarray(
        (i["enc_bih"] + i["enc_bhh"])[idx].reshape(4, 64).T.astype(np.float32))

    emb_wT = np.ascontiguousarray(i["emb_W"].T.astype(np.float32))
    emb_b = i["emb_b"].astype(np.float32).reshape(H, 1)

    we_d = i["attn_W"][:, :H].T.astype(np.float32)       # [64, 96]
    wh_d = np.zeros((H + 1, L), np.float32)
    wh_d[:H] = i["attn_W"][:, H:].T.astype(np.float32)
    wh_d[H] = i["attn_b"].astype(np.float32)

    comb_W1 = i["comb_W"][:, :H].astype(np.float32)
    comb_W2 = i["comb_W"][:, H:].astype(np.float32)
    dec_Wih = i["dec_Wih"].astype(np.float32)
    wie = pack(dec_Wih @ comb_W1)
    wic = pack(dec_Wih @ comb_W2)
    dec_whh = pack(i["dec_Whh"])
    dec_b_full = (i["dec_bih"] + i["dec_bhh"] + dec_Wih @ i["comb_b"])
    dec_b = np.ascontiguousarray(
        dec_b_full[idx].reshape(4, 64).T.astype(np.float32))

    out_wT = np.ascontiguousarray(i["out_W"].T.astype(np.float32))
    out_b = i["out_b"].astype(np.float32).reshape(C, 1)
    emb_whT = np.ascontiguousarray(
        (i["emb_W"].astype(np.float32) @ i["out_W"].astype(np.float32)).T)
    emb_bh = (i["emb_W"].astype(np.float32) @ i["out_b"].astype(np.float32)
              + i["emb_b"].astype(np.float32)).reshape(H, 1)

    return dict(enc_wih=enc_wih, enc_whh=enc_whh, enc_b=enc_b,
                emb_wT=emb_wT, emb_b=emb_b, attn_we=np.ascontiguousarray(we_d),
                attn_wh=np.ascontiguousarray(wh_d), wie=wie, wic=wic,
                dec_whh=dec_whh, dec_b=dec_b, out_wT=out_wT, out_b=out_b,
                emb_whT=emb_whT, emb_bh=emb_bh)


def kernel(**inputs):
    x_enc = np.asarray(inputs["x_enc"], np.float32)
    seq_last = x_enc[:, -1:, :]                       # [B, 1, C]
    x = x_enc - seq_last                              # [B, L, C]

    weights = _prep_weights({k: np.asarray(v) for k, v in inputs.items()
                             if k not in ("x_enc", "x_mark_enc", "x_dec",
                                          "x_mark_dec")})

    core_ids = list(range(NCORES))
    in_maps = []
    for ci in core_ids:
        xs = x[ci * BS : (ci + 1) * BS]               # [BS, L, C]
        xT = np.ascontiguousarray(xs.transpose(1, 2, 0))  # [L, C, BS]
        m = dict(weights)
        m["xT"] = xT
        in_maps.append(m)

    nc = _get_nc()
    res = run_bass_kernel_spmd(nc, in_maps, core_ids)
    global LAST_RESULTS
    LAST_RESULTS = res

    out = np.empty((B, T, C), np.float32)
    for ci in core_ids:
        p = res.results[ci]["preds"]                  # [T, C, BS]
        out[ci * BS : (ci + 1) * BS] = p.transpose(2, 0, 1)
    out += seq_last
    return out



# revision 16
# speedup vs baseline: 1.0945x; 1.0945x over previous
"""Attn_LSTM Trainium2 kernel — 8-core data-parallel Bass/Tile implementation.

Model (per reference): 1-layer LSTM encoder over L=96 steps, then T=24
attention-decoder steps. B=4096 sharded 512/core across 8 NeuronCores;
weights replicated.

Device-side design:
  * All matmul operands are bf16 (PSUM accumulation stays fp32): 4x PE rate
    vs fp32. Gates are computed in (i,f)/(g,o) PAIRS of 64 -> one [128,512]
    matmul per pair with K-stacked inputs ([h;x] K=72 encoder,
    [emb;ctx] K=128 + h K=64 decoder), quartering matmul count.
  * tanh-only activations: sigmoid(z) = 0.5*(1+tanh(z/2)). States are kept
    scaled (h2=2h, c2=2c) so every 0.5 folds into weights or the fused
    scalar_tensor_tensor cell ops; each LSTM cell is 3 scalar-engine tanh
    calls + 4 fused DVE ops.
  * Attention context via a Horner-form affine scan (tensor_tensor_scan,
    fp32 internal state): S_l = r_l*S_{l-1} + enc_l with r_l =
    exp(z_{l-1}-z_l) from PE-computed DIFFERENCED attention weights; a
    -1e30 in the difference-bias column 0 makes r=0 there, resetting the
    scan at every (h-row, chunk) boundary. A ones-row appended to enc
    yields the softmax denominator from the same scan. ONE bf16 DVE pass
    over enc per (step, chunk) instead of mul+reduce.
  * The local walrus build accepts at most ONE semaphore wait per
    instruction; legalize_waits() splits extra waits onto same-engine NoOps.
"""

import numpy as np
import ml_dtypes

import concourse.bass as bass
import concourse.tile as tile
from concourse import mybir
from concourse.masks import make_identity
from concourse.bass_utils import run_bass_kernel_spmd

H = 64
C = 8
L = 96
T = 24
B = 4096
NCORES = 8
BS = B // NCORES          # 512 batch per core
NCH = BS // 128           # 4 partition chunks per core

F32 = mybir.dt.float32
BF16 = mybir.dt.bfloat16
NPBF = ml_dtypes.bfloat16
AF = mybir.ActivationFunctionType
ALU = mybir.AluOpType


def _legalize_waits(nc):
    """This walrus build rejects >1 sem wait per instruction; split extras
    onto same-engine NoOps placed immediately before."""
    cnt = 0
    for bb in nc.main_func.blocks:
        new = []
        for inst in bb.instructions:
            si = inst.sync_info
            if si is not None and len(si.on_wait) > 1:
                waits = list(si.on_wait)
                for w in waits[:-1]:
                    nop = mybir.InstNoOp(name=f"wsplit-{cnt}", ins=[], outs=[])
                    cnt += 1
                    nop.engine = inst.engine
                    nop.sync_info = mybir.SyncInfo(on_wait=[w], on_update=[])
                    new.append(nop)
                inst.sync_info = mybir.SyncInfo(
                    on_wait=[waits[-1]], on_update=list(si.on_update))
            new.append(inst)
        bb.instructions = new
    return cnt


def _tts_raw(nc, eng, out, data0, data1, initial, op0, op1):
    """tensor_tensor_scan without the 2D-shape assert: the recurrence chains
    across free dims, which we exploit (r=0 at row starts resets the state)."""
    return eng.add_instruction(
        mybir.InstTensorScalarPtr(
            name=nc.get_next_instruction_name(),
            is_tensor_tensor_scan=True,
            is_scalar_tensor_tensor=True,
            op0=op0,
            op1=op1,
            ins=[
                eng.lower_ap(data0),
                eng.lower_ap_or_imm(initial),
                eng.lower_ap(data1),
            ],
            outs=[eng.lower_ap(out)],
        )
    )


def _build_program():
    nc = bass.Bass("TRN2", target_bir_lowering=False, debug=False,
                   num_devices=NCORES)

    def din(name, shape, dt=BF16):
        return nc.dram_tensor(name, list(shape), dt, kind="ExternalInput").ap()

    xT = din("xT", (L, C, BS))                  # normalized, transposed, bf16
    enc_w = din("enc_w", (72, 2, 128))          # rows 0:64=.5*Whh_p.T, 64:72=Wih_p.T
    dec_ec = din("dec_ec", (128, 2, 128))       # rows 0:64=wie_p.T, 64:128=.5*wic_p.T
    dec_hh = din("dec_hh", (64, 2, 128))        # .5*dec_Whh_p.T
    we_d = din("we_d", (H, L))                  # differenced attn emb-part
    wh_d = din("wh_d", (H + 1, L))              # differenced .5*attn h-part + bias row
    w_emb = din("w_emb", (H, H))                # .5*(emb_W@out_W).T
    w_out = din("w_out", (H, C))                # .5*out_W.T
    b_enc = din("b_enc", (128, 2), F32)         # act biases per pair
    b_dec = din("b_dec", (128, 2), F32)
    sc_go = din("sc_go", (128, 1), F32)         # act scale for (g,o) pair: 1 / .5
    emb_bh = din("emb_bh", (H, 1), F32)         # emb_W@out_b + emb_b
    emb0 = din("emb0", (H, 1), F32)             # relu(emb_b)  (t=0 embedding)
    out_b = din("out_b", (C, 1), F32)

    preds = nc.dram_tensor("preds", [T, C, BS], F32, kind="ExternalOutput").ap()

    with tile.TileContext(nc) as tc:
        with (
            tc.tile_pool(name="state", bufs=1) as st,
            tc.tile_pool(name="xin", bufs=2) as xin,
            tc.tile_pool(name="scan", bufs=2) as scp,
            tc.tile_pool(name="outp", bufs=2) as outp,
            tc.tile_pool(name="gps", bufs=1, space="PSUM") as gps,
            tc.tile_pool(name="tps", bufs=2, space="PSUM") as tps,
            tc.tile_pool(name="mps", bufs=1, space="PSUM") as mps,
        ):
            # ---------- persistent tiles ----------
            ident_f = st.tile([128, 128], F32)
            make_identity(nc, ident_f[:])
            ident = st.tile([128, 128], BF16)
            nc.scalar.copy(ident[:], ident_f[:])

            w_enc_sb = st.tile([72, 2, 128], BF16)
            w_ec_sb = st.tile([128, 2, 128], BF16)
            w_hh_sb = st.tile([64, 2, 128], BF16)
            w_we_sb = st.tile([H, L], BF16)
            w_wh_sb = st.tile([H + 1, L], BF16)
            w_emb_sb = st.tile([H, H], BF16)
            w_out_sb = st.tile([H, C], BF16)
            b_enc_sb = st.tile([128, 2], F32)
            b_dec_sb = st.tile([128, 2], F32)
            sc_go_sb = st.tile([128, 1], F32)
            emb_bh_sb = st.tile([H, 1], F32)
            emb0_sb = st.tile([H, 1], F32)
            out_b_sb = st.tile([C, 1], F32)
            for tl, ap in ((w_enc_sb, enc_w), (w_ec_sb, dec_ec),
                           (w_hh_sb, dec_hh), (w_we_sb, we_d),
                           (w_wh_sb, wh_d), (w_emb_sb, w_emb),
                           (w_out_sb, w_out), (b_enc_sb, b_enc),
                           (b_dec_sb, b_dec), (sc_go_sb, sc_go),
                           (emb_bh_sb, emb_bh), (emb0_sb, emb0),
                           (out_b_sb, out_b)):
                nc.gpsimd.dma_start(tl[:], ap[:])

            # recurrent state (scaled): h2=2h (bf16), c2=2c (fp32);
            # h_T row 64 is a ones-row feeding the differenced attn bias.
            # c2/tc live at partitions 64:128 so the two-tensor DVE ops have
            # partition-aligned inputs (f'/o' sit at rows 64:128 of the pair
            # tiles); outputs may shift partitions freely.
            h_T = st.tile([H + 1, BS], BF16)
            c2b = st.tile([128, BS], F32)
            nc.vector.memset(h_T[:], 0.0)
            nc.vector.memset(c2b[64:128, :], 0.0)
            nc.vector.memset(h_T[H : H + 1, :], 1.0)

            # encoder outputs (+ ones row): [b, chunk, h(65), l], bf16
            enc_plus = st.tile([128, NCH, H + 1, L], BF16)
            nc.vector.memset(enc_plus[:, :, H, :], 1.0)

            # encoder combined rhs: rows 0:64 = h2, 64:72 = x (ping-pong)
            xh = [st.tile([72, BS], BF16, name=f"xh{p}") for p in range(2)]
            for p in range(2):
                nc.vector.memset(xh[p][0:H, :], 0.0)   # h2_0 = 0

            # act outputs + cell temps
            if_sb = st.tile([128, BS], BF16)
            go_sb = st.tile([128, BS], BF16)
            f2_sb = st.tile([128, BS], BF16)   # used rows 64:128 (f'+1, Pool)
            t1_sb = st.tile([H, BS], F32)
            t2_sb = st.tile([H, BS], F32)
            tc_sb = st.tile([128, BS], BF16)   # used rows 64:128

            # decoder tiles
            ec_sb = st.tile([128, BS], BF16)       # rows 0:64 emb, 64:128 ctx2
            r_sb = st.tile([128, NCH, L], BF16)
            rec_sb = st.tile([128, NCH], F32)
            ctx_ch = st.tile([128, NCH, H], BF16)

            # PSUM
            gate_ps = [gps.tile([128, BS], F32, tag=f"gp{p}", name=f"gp{p}")
                       for p in range(2)]

            def lstm_cell(bias_tile):
                """gate pairs (psum) -> h2/c2 update; h2 written to h_out."""
                # (g,o) first, then (i,f): t2/t1 start as soon as possible
                nc.scalar.activation(go_sb[:], gate_ps[1][:], AF.Tanh,
                                     bias=bias_tile[:, 1:2],
                                     scale=sc_go_sb[:, 0:1])
                nc.scalar.activation(if_sb[:], gate_ps[0][:], AF.Tanh,
                                     bias=bias_tile[:, 0:1], scale=0.5)
                # t2 = (f'+1)*c2 ; t1 = (i'+1)*tg ; c2 = .5*t2 + t1
                # Pool has no STT opcode: split t2 into ts + tt (both Pool-legal)
                nc.gpsimd.tensor_scalar(
                    out=f2_sb[H:128, :], in0=if_sb[H:128, :],
                    scalar1=1.0, scalar2=None, op0=ALU.add)
                nc.gpsimd.tensor_mul(t2_sb[:], f2_sb[H:128, :], c2b[H:128, :])
                nc.vector.scalar_tensor_tensor(
                    out=t1_sb[:], in0=if_sb[0:H, :], scalar=1.0,
                    in1=go_sb[0:H, :], op0=ALU.add, op1=ALU.mult)
                nc.vector.scalar_tensor_tensor(
                    out=c2b[H:128, :], in0=t2_sb[:], scalar=0.5,
                    in1=t1_sb[:], op0=ALU.mult, op1=ALU.add)
                nc.scalar.activation(tc_sb[H:128, :], c2b[H:128, :],
                                     AF.Tanh, scale=0.5)

            def h2_out(dst):
                # h2 = (o'+1)*tanh(c)  (bf16)
                nc.vector.scalar_tensor_tensor(
                    out=dst, in0=go_sb[H:128, :], scalar=1.0,
                    in1=tc_sb[H:128, :], op0=ALU.add, op1=ALU.mult)

            # ------------------ encoder ------------------
            nc.sync.dma_start(xh[0][H:72, :], xT[0])
            for l in range(L):
                if l + 1 < L:
                    nc.sync.dma_start(xh[(l + 1) % 2][H:72, :], xT[l + 1])
                for p in range(2):
                    nc.tensor.matmul(gate_ps[p][:], w_enc_sb[:, p, :],
                                     xh[l % 2][:], start=True, stop=True)
                lstm_cell(b_enc_sb)
                holder = h_T if l == L - 1 else xh[(l + 1) % 2]
                h2_out(holder[0:H, :])
                # store h2 (transposed back to [b, h]) into enc_plus[:,:,0:H,l]
                tp = tps.tile([128, NCH, H], BF16, tag="tp")
                for ci in range(NCH):
                    nc.tensor.transpose(tp[:, ci, :],
                                        holder[0:H, 128 * ci : 128 * (ci + 1)],
                                        ident[0:H, 0:H])
                nc.vector.tensor_scalar(
                    out=enc_plus[:, :, 0:H, l], in0=tp[:],
                    scalar1=0.0, scalar2=None, op0=ALU.add)

            # ------------------ decoder ------------------
            for t in range(T):
                # embedding into ec rows 0:64 (bf16)
                if t == 0:
                    nc.vector.tensor_scalar(
                        out=ec_sb[0:H, :],
                        in0=emb0_sb[:, 0:1].broadcast_to((H, BS)),
                        scalar1=0.0, scalar2=None, op0=ALU.add)
                else:
                    emb_ps = mps.tile([H, BS], F32, tag="emb")
                    nc.tensor.matmul(emb_ps[:], w_emb_sb[:], h_T[0:H, :],
                                     start=True, stop=True)
                    nc.scalar.activation(ec_sb[0:H, :], emb_ps[:], AF.Relu,
                                         bias=emb_bh_sb[:, 0:1])

                # differenced attention scores -> exp -> scan ratios
                zd_ps = mps.tile([128, NCH, L], F32, tag="zd")
                for ci in range(NCH):
                    sl = slice(128 * ci, 128 * (ci + 1))
                    nc.tensor.matmul(zd_ps[:, ci, :], ec_sb[0:H, sl],
                                     w_we_sb[:], start=True, stop=False)
                    nc.tensor.matmul(zd_ps[:, ci, :], h_T[:, sl],
                                     w_wh_sb[:], start=False, stop=True)
                nc.scalar.activation(r_sb[:], zd_ps[:], AF.Exp)

                # Horner scan per chunk; last column = weighted sums
                ctxT_ps = mps.tile([H, BS], BF16, tag="ctxT")
                for ci in range(NCH):
                    S_t = scp.tile([128, H + 1, L], BF16, tag=f"S{ci % 2}")
                    _tts_raw(nc, nc.vector, S_t[:],
                             r_sb[:, ci, :].unsqueeze(1).broadcast_to(
                                 (128, H + 1, L)),
                             enc_plus[:, ci], 0.0, ALU.mult, ALU.add)
                    nc.vector.reciprocal(rec_sb[:, ci : ci + 1],
                                         S_t[:, H, L - 1 : L])
                    nc.vector.tensor_scalar(
                        out=ctx_ch[:, ci, :], in0=S_t[:, 0:H, L - 1],
                        scalar1=rec_sb[:, ci : ci + 1], scalar2=None,
                        op0=ALU.mult)
                    nc.tensor.transpose(ctxT_ps[:, 128 * ci : 128 * (ci + 1)],
                                        ctx_ch[:, ci, :], ident[:])
                nc.scalar.copy(ec_sb[H:128, :], ctxT_ps[:])

                # decoder LSTM cell (comb layer folded into gate weights)
                for p in range(2):
                    nc.tensor.matmul(gate_ps[p][:], w_ec_sb[:, p, :],
                                     ec_sb[:], start=True, stop=False)
                    nc.tensor.matmul(gate_ps[p][:], w_hh_sb[:, p, :],
                                     h_T[0:H, :], start=False, stop=True)
                lstm_cell(b_dec_sb)
                h2_out(h_T[0:H, :])

                # prediction -> output store
                pred_ps = mps.tile([C, BS], F32, tag="pred")
                nc.tensor.matmul(pred_ps[:], w_out_sb[:], h_T[0:H, :],
                                 start=True, stop=True)
                po = outp.tile([C, BS], F32, tag="po")
                nc.scalar.activation(po[:], pred_ps[:], AF.Identity,
                                     bias=out_b_sb[:, 0:1])
                nc.sync.dma_start(preds[t], po[:])

    _legalize_waits(nc)
    return nc


_NC_CACHE = []


def _get_nc():
    if not _NC_CACHE:
        _NC_CACHE.append(_build_program())
    return _NC_CACHE[0]


def _bf(x):
    return np.ascontiguousarray(np.asarray(x, np.float32).astype(NPBF))


def _prep_weights(i):
    """Host-side packing. Gate pairs: p0=(i,f), p1=(g,o) in pytorch row order."""
    Wih = np.asarray(i["enc_Wih"], np.float32)
    Whh = np.asarray(i["enc_Whh"], np.float32)
    be = np.asarray(i["enc_bih"] + i["enc_bhh"], np.float32)

    enc_w = np.zeros((72, 2, 128), np.float32)
    for p in range(2):
        r = slice(128 * p, 128 * (p + 1))
        enc_w[0:64, p, :] = 0.5 * Whh[r].T
        enc_w[64:72, p, :] = Wih[r].T

    emb_W = np.asarray(i["emb_W"], np.float32)
    emb_b = np.asarray(i["emb_b"], np.float32)
    attn_W = np.asarray(i["attn_W"], np.float32)
    attn_b = np.asarray(i["attn_b"], np.float32)
    comb_W = np.asarray(i["comb_W"], np.float32)
    comb_b = np.asarray(i["comb_b"], np.float32)
    dWih = np.asarray(i["dec_Wih"], np.float32)
    dWhh = np.asarray(i["dec_Whh"], np.float32)
    bd = (np.asarray(i["dec_bih"] + i["dec_bhh"], np.float32)
          + dWih @ comb_b)
    out_W = np.asarray(i["out_W"], np.float32)
    out_bv = np.asarray(i["out_b"], np.float32)

    wie = dWih @ comb_W[:, :H]
    wic = dWih @ comb_W[:, H:]
    dec_ec = np.zeros((128, 2, 128), np.float32)
    dec_hh = np.zeros((64, 2, 128), np.float32)
    for p in range(2):
        r = slice(128 * p, 128 * (p + 1))
        dec_ec[0:64, p, :] = wie[r].T
        dec_ec[64:128, p, :] = 0.5 * wic[r].T
        dec_hh[:, p, :] = 0.5 * dWhh[r].T

    We = attn_W[:, :H].T
    Wh = attn_W[:, H:].T
    we_d = np.zeros((H, L), np.float32)
    wh_d = np.zeros((H + 1, L), np.float32)
    we_d[:, 1:] = We[:, :-1] - We[:, 1:]
    wh_d[0:H, 1:] = 0.5 * (Wh[:, :-1] - Wh[:, 1:])
    wh_d[H, 1:] = attn_b[:-1] - attn_b[1:]
    wh_d[H, 0] = -1e30

    def bias_pack(b):
        out = np.zeros((128, 2), np.float32)
        out[:, 0] = 0.5 * b[0:128]
        out[0:64, 1] = b[128:192]
        out[64:128, 1] = 0.5 * b[192:256]
        return out

    sc_go = np.zeros((128, 1), np.float32)
    sc_go[0:64] = 1.0
    sc_go[64:128] = 0.5

    return dict(
        enc_w=_bf(enc_w), dec_ec=_bf(dec_ec), dec_hh=_bf(dec_hh),
        we_d=_bf(we_d), wh_d=_bf(wh_d),
        w_emb=_bf(0.5 * (emb_W @ out_W).T), w_out=_bf(0.5 * out_W.T),
        b_enc=bias_pack(be), b_dec=bias_pack(bd), sc_go=sc_go,
        emb_bh=(emb_W @ out_bv + emb_b).reshape(H, 1).astype(np.float32),
        emb0=np.maximum(emb_b, 0.0).reshape(H, 1).astype(np.float32),
        out_b=out_bv.reshape(C, 1).astype(np.float32),
    )


def kernel(**inputs):
    x_enc = np.asarray(inputs["x_enc"], np.float32)
    seq_last = x_enc[:, -1:, :]                       # [B, 1, C]
    x = x_enc - seq_last                              # [B, L, C]

    weights = _prep_weights({k: np.asarray(v) for k, v in inputs.items()
                             if k not in ("x_enc", "x_mark_enc", "x_dec",
                                          "x_mark_dec")})

    core_ids = list(range(NCORES))
    in_maps = []
    for ci in core_ids:
        xs = x[ci * BS : (ci + 1) * BS]               # [BS, L, C]
        xTc = np.ascontiguousarray(
            xs.transpose(1, 2, 0).astype(NPBF))       # [L, C, BS] bf16
        m = dict(weights)
        m["xT"] = xTc
        in_maps.append(m)

    nc = _get_nc()
    res = run_bass_kernel_spmd(nc, in_maps, core_ids)
    global LAST_RESULTS
    LAST_RESULTS = res

    out = np.empty((B, T, C), np.float32)
    for ci in core_ids:
        p = res.results[ci]["preds"]                  # [T, C, BS]
        out[ci * BS : (ci + 1) * BS] = p.transpose(2, 0, 1)
    out += seq_last
    return out


# revision 17
# speedup vs baseline: 1.9304x; 1.7637x over previous
"""Attn_LSTM Trainium2 kernel — 8-core data-parallel Bass/Tile implementation.

Model (per reference): 1-layer LSTM encoder over L=96 steps, then T=24
attention-decoder steps. B=4096 sharded 512/core across 8 NeuronCores;
weights replicated.

Device-side design (driven by measured engine rates):
  * PE matmuls all-bf16 (fp32 PSUM accumulation), gates paired (i,f)/(g,o)
    into [128,512] matmuls with K-stacked inputs ([h;x] K=72 encoder,
    [emb;ctx] K=128 + h K=64 decoder).
  * DVE: only TensorTensor (1 elem/cyc/lane fp32, 2/cyc pure-bf16) and
    TensorScalar (2/cyc) ops — scalar_tensor_tensor and tensor_tensor_scan
    are microcoded ~8-20x slower on this DVE and are avoided. GpSimd (Pool)
    is erratic/slow and unused for compute.
  * Cell: sigmoid/tanh activations on the ACT engine (cost ~0.84ns/col,
    independent of partition count, so partition-paired gates are free);
    cell math is 4 pure-bf16 tensor_tensor ops. States bf16.
  * Attention context: softmax numerators e=exp(z) from one ACT call; then
    ctx = (sum_l e_l*enc_l)/(sum_l e_l) via ONE fused bf16 multiply over
    [128, NCH, L, H+1] (e broadcast along h; broadcasts are free) and a
    7-op bf16 binary ADD TREE over l. A ones-column at h=H yields the
    softmax denominator from the same pass.
  * The local walrus build accepts at most ONE semaphore wait per
    instruction; legalize_waits() splits extra waits onto same-engine NoOps.
"""

import numpy as np
import ml_dtypes

import concourse.bass as bass
import concourse.tile as tile
from concourse import mybir
from concourse.masks import make_identity
from concourse.bass_utils import run_bass_kernel_spmd

H = 64
C = 8
L = 96
T = 24
B = 4096
NCORES = 8
BS = B // NCORES          # 512 batch per core
NCH = BS // 128           # 4 partition chunks per core

F32 = mybir.dt.float32
BF16 = mybir.dt.bfloat16
NPBF = ml_dtypes.bfloat16
AF = mybir.ActivationFunctionType
ALU = mybir.AluOpType


def _legalize_waits(nc):
    """This walrus build rejects >1 sem wait per instruction; split extras
    onto same-engine NoOps placed immediately before."""
    cnt = 0
    for bb in nc.main_func.blocks:
        new = []
        for inst in bb.instructions:
            si = inst.sync_info
            if si is not None and len(si.on_wait) > 1:
                waits = list(si.on_wait)
                for w in waits[:-1]:
                    nop = mybir.InstNoOp(name=f"wsplit-{cnt}", ins=[], outs=[])
                    cnt += 1
                    nop.engine = inst.engine
                    nop.sync_info = mybir.SyncInfo(on_wait=[w], on_update=[])
                    new.append(nop)
                inst.sync_info = mybir.SyncInfo(
                    on_wait=[waits[-1]], on_update=list(si.on_update))
            new.append(inst)
        bb.instructions = new
    return cnt


def _tts_raw(nc, eng, out, data0, data1, initial, op0, op1):
    """tensor_tensor_scan without the 2D-shape assert (kept for probes)."""
    return eng.add_instruction(
        mybir.InstTensorScalarPtr(
            name=nc.get_next_instruction_name(),
            is_tensor_tensor_scan=True,
            is_scalar_tensor_tensor=True,
            op0=op0,
            op1=op1,
            ins=[
                eng.lower_ap(data0),
                eng.lower_ap_or_imm(initial),
                eng.lower_ap(data1),
            ],
            outs=[eng.lower_ap(out)],
        )
    )


def _build_program():
    nc = bass.Bass("TRN2", target_bir_lowering=False, debug=False,
                   num_devices=NCORES)

    def din(name, shape, dt=BF16):
        return nc.dram_tensor(name, list(shape), dt, kind="ExternalInput").ap()

    xT = din("xT", (L, C, BS))                  # normalized, transposed, bf16
    enc_w = din("enc_w", (72, 2, 128))          # rows 0:64=Whh_p.T, 64:72=Wih_p.T
    dec_ec = din("dec_ec", (128, 2, 128))       # rows 0:64=wie_p.T, 64:128=wic_p.T
    dec_hh = din("dec_hh", (64, 2, 128))        # dec_Whh_p.T
    w_we = din("w_we", (H, L))                  # attn emb-part We.T
    w_wh = din("w_wh", (H + 1, L))              # attn h-part Wh.T + bias row
    w_emb = din("w_emb", (H, H))                # (emb_W@out_W).T
    w_out = din("w_out", (H, C))                # out_W.T
    b_enc = din("b_enc", (128, 2), F32)         # act biases per pair
    b_dec = din("b_dec", (128, 2), F32)
    emb_bh = din("emb_bh", (H, 1), F32)         # emb_W@out_b + emb_b
    emb0 = din("emb0", (H, 1), F32)             # relu(emb_b)  (t=0 embedding)
    out_b = din("out_b", (C, 1), F32)

    preds = nc.dram_tensor("preds", [T, C, BS], F32, kind="ExternalOutput").ap()

    with tile.TileContext(nc) as tc:
        with (
            tc.tile_pool(name="state", bufs=1) as st,
            tc.tile_pool(name="outp", bufs=2) as outp,
            tc.tile_pool(name="gps", bufs=1, space="PSUM") as gps,
            tc.tile_pool(name="tps", bufs=2, space="PSUM") as tps,
            tc.tile_pool(name="mps", bufs=1, space="PSUM") as mps,
        ):
            # ---------- persistent tiles ----------
            ident_f = st.tile([128, 128], F32)
            make_identity(nc, ident_f[:])
            ident = st.tile([128, 128], BF16)
            nc.scalar.copy(ident[:], ident_f[:])

            w_enc_sb = st.tile([72, 2, 128], BF16)
            w_ec_sb = st.tile([128, 2, 128], BF16)
            w_hh_sb = st.tile([64, 2, 128], BF16)
            w_we_sb = st.tile([H, L], BF16)
            w_wh_sb = st.tile([H + 1, L], BF16)
            w_emb_sb = st.tile([H, H], BF16)
            w_out_sb = st.tile([H, C], BF16)
            b_enc_sb = st.tile([128, 2], F32)
            b_dec_sb = st.tile([128, 2], F32)
            emb_bh_sb = st.tile([H, 1], F32)
            emb0_sb = st.tile([H, 1], F32)
            out_b_sb = st.tile([C, 1], F32)
            for tl, ap in ((w_enc_sb, enc_w), (w_ec_sb, dec_ec),
                           (w_hh_sb, dec_hh), (w_we_sb, w_we),
                           (w_wh_sb, w_wh), (w_emb_sb, w_emb),
                           (w_out_sb, w_out), (b_enc_sb, b_enc),
                           (b_dec_sb, b_dec), (emb_bh_sb, emb_bh),
                           (emb0_sb, emb0), (out_b_sb, out_b)):
                nc.gpsimd.dma_start(tl[:], ap[:])

            # recurrent state: h (bf16) with ones row 64 (attn bias);
            # c lives at partitions 64:128 so the two-input DVE ops have
            # partition-aligned operands (f/o sit at rows 64:128 of the
            # pair tiles); outputs may shift partitions freely.
            h_T = st.tile([H + 1, BS], BF16)
            cb = st.tile([128, BS], BF16)      # c at rows 64:128
            nc.vector.memset(h_T[:], 0.0)
            nc.vector.memset(cb[64:128, :], 0.0)
            nc.vector.memset(h_T[H : H + 1, :], 1.0)

            # encoder outputs [b, chunk, l, h(65)] bf16; col H = ones
            enc_plus = st.tile([128, NCH, L, H + 1], BF16)
            nc.vector.memset(enc_plus[:, :, :, H], 1.0)

            # encoder combined rhs: rows 0:64 = h, 64:72 = x (ping-pong)
            xh = [st.tile([72, BS], BF16, name=f"xh{p}") for p in range(2)]
            for p in range(2):
                nc.vector.memset(xh[p][0:H, :], 0.0)

            # act outputs + cell temps
            if_sb = st.tile([128, BS], BF16)
            go_sb = st.tile([128, BS], BF16)
            t1_sb = st.tile([H, BS], BF16)
            t2_sb = st.tile([H, BS], BF16)
            tc_sb = st.tile([128, BS], BF16)   # used rows 64:128

            # decoder tiles
            ec_sb = st.tile([128, BS], BF16)       # rows 0:64 emb, 64:128 ctx
            e_sb = st.tile([128, NCH, L], BF16)
            rec_sb = st.tile([128, NCH], F32)
            ctx_ch = st.tile([128, NCH, H], BF16)
            P = st.tile([128, NCH, L, H + 1], BF16)
            qa = st.tile([128, NCH, 48, H + 1], BF16)
            qb = st.tile([128, NCH, 24, H + 1], BF16)
            qc = st.tile([128, NCH, 12, H + 1], BF16)
            qd = st.tile([128, NCH, 6, H + 1], BF16)
            qe = st.tile([128, NCH, 3, H + 1], BF16)
            Rt = st.tile([128, NCH, H + 1], BF16)
            St = st.tile([128, NCH, H + 1], BF16)

            # PSUM
            gate_ps = [gps.tile([128, BS], F32, tag=f"gp{p}", name=f"gp{p}")
                       for p in range(2)]

            def lstm_cell(bias_tile):
                """pair PSUMs -> activations -> c/h update (h into dst)."""
                nc.scalar.activation(go_sb[0:H, :], gate_ps[1][0:H, :],
                                     AF.Tanh, bias=bias_tile[0:H, 1:2])
                nc.scalar.activation(if_sb[:], gate_ps[0][:], AF.Sigmoid,
                                     bias=bias_tile[:, 0:1])
                nc.scalar.activation(go_sb[H:128, :], gate_ps[1][H:128, :],
                                     AF.Sigmoid, bias=bias_tile[H:128, 1:2])
                nc.vector.tensor_mul(t1_sb[:], if_sb[0:H, :], go_sb[0:H, :])
                nc.vector.tensor_mul(t2_sb[:], if_sb[H:128, :], cb[H:128, :])
                nc.vector.tensor_add(cb[H:128, :], t1_sb[:], t2_sb[:])
                nc.scalar.activation(tc_sb[H:128, :], cb[H:128, :], AF.Tanh)

            def h_out(dst):
                nc.vector.tensor_mul(dst, go_sb[H:128, :], tc_sb[H:128, :])

            # ------------------ encoder ------------------
            nc.sync.dma_start(xh[0][H:72, :], xT[0])
            for l in range(L):
                if l + 1 < L:
                    nc.sync.dma_start(xh[(l + 1) % 2][H:72, :], xT[l + 1])
                for p in range(2):
                    nc.tensor.matmul(gate_ps[p][:], w_enc_sb[:, p, :],
                                     xh[l % 2][:], start=True, stop=True)
                lstm_cell(b_enc_sb)
                holder = h_T if l == L - 1 else xh[(l + 1) % 2]
                h_out(holder[0:H, :])
                # store h (transposed back to [b, h]) into enc_plus[:,:,l,0:H]
                tp = tps.tile([128, NCH, H], BF16, tag="tp")
                for ci in range(NCH):
                    nc.tensor.transpose(tp[:, ci, :],
                                        holder[0:H, 128 * ci : 128 * (ci + 1)],
                                        ident[0:H, 0:H])
                nc.vector.tensor_scalar(
                    out=enc_plus[:, :, l, 0:H], in0=tp[:],
                    scalar1=0.0, scalar2=None, op0=ALU.add)

            # ------------------ decoder ------------------
            for t in range(T):
                # embedding into ec rows 0:64 (bf16)
                if t == 0:
                    nc.vector.tensor_scalar(
                        out=ec_sb[0:H, :],
                        in0=emb0_sb[:, 0:1].broadcast_to((H, BS)),
                        scalar1=0.0, scalar2=None, op0=ALU.add)
                else:
                    emb_ps = mps.tile([H, BS], F32, tag="emb")
                    nc.tensor.matmul(emb_ps[:], w_emb_sb[:], h_T[0:H, :],
                                     start=True, stop=True)
                    nc.scalar.activation(ec_sb[0:H, :], emb_ps[:], AF.Relu,
                                         bias=emb_bh_sb[:, 0:1])

                # attention scores -> e = exp(z)
                zd_ps = mps.tile([128, NCH, L], F32, tag="zd")
                for ci in range(NCH):
                    sl = slice(128 * ci, 128 * (ci + 1))
                    nc.tensor.matmul(zd_ps[:, ci, :], ec_sb[0:H, sl],
                                     w_we_sb[:], start=True, stop=False)
                    nc.tensor.matmul(zd_ps[:, ci, :], h_T[:, sl],
                                     w_wh_sb[:], start=False, stop=True)
                nc.scalar.activation(e_sb[:], zd_ps[:], AF.Exp)

                # ctx: one fused bf16 multiply + bf16 add-tree over l
                e_bc = e_sb[:].unsqueeze(3).broadcast_to((128, NCH, L, H + 1))
                nc.vector.tensor_mul(P[:], enc_plus[:], e_bc)
                nc.vector.tensor_add(qa[:], P[:, :, 0:48], P[:, :, 48:96])
                nc.vector.tensor_add(qb[:], qa[:, :, 0:24], qa[:, :, 24:48])
                nc.vector.tensor_add(qc[:], qb[:, :, 0:12], qb[:, :, 12:24])
                nc.vector.tensor_add(qd[:], qc[:, :, 0:6], qc[:, :, 6:12])
                nc.vector.tensor_add(qe[:], qd[:, :, 0:3], qd[:, :, 3:6])
                nc.vector.tensor_add(Rt[:], qe[:, :, 0, :], qe[:, :, 1, :])
                nc.vector.tensor_add(St[:], Rt[:], qe[:, :, 2, :])

                ctxT_ps = mps.tile([H, BS], BF16, tag="ctxT")
                for ci in range(NCH):
                    nc.vector.reciprocal(rec_sb[:, ci : ci + 1],
                                         St[:, ci, H : H + 1])
                    nc.vector.tensor_scalar(
                        out=ctx_ch[:, ci, :], in0=St[:, ci, 0:H],
                        scalar1=rec_sb[:, ci : ci + 1], scalar2=None,
                        op0=ALU.mult)
                    nc.tensor.transpose(ctxT_ps[:, 128 * ci : 128 * (ci + 1)],
                                        ctx_ch[:, ci, :], ident[:])
                nc.scalar.copy(ec_sb[H:128, :], ctxT_ps[:])

                # decoder LSTM cell (comb layer folded into gate weights)
                for p in range(2):
                    nc.tensor.matmul(gate_ps[p][:], w_ec_sb[:, p, :],
                                     ec_sb[:], start=True, stop=False)
                    nc.tensor.matmul(gate_ps[p][:], w_hh_sb[:, p, :],
                                     h_T[0:H, :], start=False, stop=True)
                lstm_cell(b_dec_sb)
                h_out(h_T[0:H, :])

                # prediction -> output store
                pred_ps = mps.tile([C, BS], F32, tag="pred")
                nc.tensor.matmul(pred_ps[:], w_out_sb[:], h_T[0:H, :],
                                 start=True, stop=True)
                po = outp.tile([C, BS], F32, tag="po")
                nc.scalar.activation(po[:], pred_ps[:], AF.Identity,
                                     bias=out_b_sb[:, 0:1])
                nc.sync.dma_start(preds[t], po[:])

    _legalize_waits(nc)
    return nc


_NC_CACHE = []


def _get_nc():
    if not _NC_CACHE:
        _NC_CACHE.append(_build_program())
    return _NC_CACHE[0]


def _bf(x):
    return np.ascontiguousarray(np.asarray(x, np.float32).astype(NPBF))


def _prep_weights(i):
    """Host-side packing. Gate pairs: p0=(i,f), p1=(g,o) in pytorch row order."""
    Wih = np.asarray(i["enc_Wih"], np.float32)
    Whh = np.asarray(i["enc_Whh"], np.float32)
    be = np.asarray(i["enc_bih"] + i["enc_bhh"], np.float32)

    enc_w = np.zeros((72, 2, 128), np.float32)
    for p in range(2):
        r = slice(128 * p, 128 * (p + 1))
        enc_w[0:64, p, :] = Whh[r].T
        enc_w[64:72, p, :] = Wih[r].T

    emb_W = np.asarray(i["emb_W"], np.float32)
    emb_b = np.asarray(i["emb_b"], np.float32)
    attn_W = np.asarray(i["attn_W"], np.float32)
    attn_b = np.asarray(i["attn_b"], np.float32)
    comb_W = np.asarray(i["comb_W"], np.float32)
    comb_b = np.asarray(i["comb_b"], np.float32)
    dWih = np.asarray(i["dec_Wih"], np.float32)
    dWhh = np.asarray(i["dec_Whh"], np.float32)
    bd = (np.asarray(i["dec_bih"] + i["dec_bhh"], np.float32)
          + dWih @ comb_b)
    out_W = np.asarray(i["out_W"], np.float32)
    out_bv = np.asarray(i["out_b"], np.float32)

    wie = dWih @ comb_W[:, :H]
    wic = dWih @ comb_W[:, H:]
    dec_ec = np.zeros((128, 2, 128), np.float32)
    dec_hh = np.zeros((64, 2, 128), np.float32)
    for p in range(2):
        r = slice(128 * p, 128 * (p + 1))
        dec_ec[0:64, p, :] = wie[r].T
        dec_ec[64:128, p, :] = wic[r].T
        dec_hh[:, p, :] = dWhh[r].T

    w_wh = np.zeros((H + 1, L), np.float32)
    w_wh[0:H] = attn_W[:, H:].T
    w_wh[H] = attn_b

    def bias_pack(b):
        out = np.zeros((128, 2), np.float32)
        out[:, 0] = b[0:128]
        out[0:64, 1] = b[128:192]
        out[64:128, 1] = b[192:256]
        return out

    return dict(
        enc_w=_bf(enc_w), dec_ec=_bf(dec_ec), dec_hh=_bf(dec_hh),
        w_we=_bf(attn_W[:, :H].T), w_wh=_bf(w_wh),
        w_emb=_bf((emb_W @ out_W).T), w_out=_bf(out_W.T),
        b_enc=bias_pack(be), b_dec=bias_pack(bd),
        emb_bh=(emb_W @ out_bv + emb_b).reshape(H, 1).astype(np.float32),
        emb0=np.maximum(emb_b, 0.0).reshape(H, 1).astype(np.float32),
        out_b=out_bv.reshape(C, 1).astype(np.float32),
    )


def kernel(**inputs):
    x_enc = np.asarray(inputs["x_enc"], np.float32)
    seq_last = x_enc[:, -1:, :]                       # [B, 1, C]
    x = x_enc - seq_last                              # [B, L, C]

    weights = _prep_weights({k: np.asarray(v) for k, v in inputs.items()
                             if k not in ("x_enc", "x_mark_enc", "x_dec",
                                          "x_mark_dec")})

    core_ids = list(range(NCORES))
    in_maps = []
    for ci in core_ids:
        xs = x[ci * BS : (ci + 1) * BS]               # [BS, L, C]
        xTc = np.ascontiguousarray(
            xs.transpose(1, 2, 0).astype(NPBF))       # [L, C, BS] bf16
        m = dict(weights)
        m["xT"] = xTc
        in_maps.append(m)

    nc = _get_nc()
    res = run_bass_kernel_spmd(nc, in_maps, core_ids)
    global LAST_RESULTS
    LAST_RESULTS = res

    out = np.empty((B, T, C), np.float32)
    for ci in core_ids:
        p = res.results[ci]["preds"]                  # [T, C, BS]
        out[ci * BS : (ci + 1) * BS] = p.transpose(2, 0, 1)
    out += seq_last
    return out


# revision 18
# speedup vs baseline: 2.2242x; 1.1522x over previous
"""Attn_LSTM Trainium2 kernel — 8-core data-parallel Bass/Tile implementation.

Model (per reference): 1-layer LSTM encoder over L=96 steps, then T=24
attention-decoder steps. B=4096 sharded 512/core across 8 NeuronCores;
weights replicated.

Device-side design (driven by measured engine rates):
  * PE matmuls all-bf16 (fp32 PSUM accumulation), gates paired (i,f)/(g,o)
    into [128,512] matmuls with K-stacked inputs ([h;x] K=72 encoder,
    [emb;ctx] K=128 + h K=64 decoder).
  * DVE: only TensorTensor (1 elem/cyc/lane fp32, 2/cyc pure-bf16) and
    TensorScalar (2/cyc) ops — scalar_tensor_tensor and tensor_tensor_scan
    are microcoded ~8-20x slower on this DVE and are avoided. GpSimd (Pool)
    is erratic/slow and unused for compute.
  * Cell: sigmoid/tanh activations on the ACT engine (cost ~0.84ns/col,
    independent of partition count, so partition-paired gates are free);
    cell math is 4 pure-bf16 tensor_tensor ops. States bf16.
  * Attention context: softmax numerators e=exp(z) from one ACT call; then
    ctx = (sum_l e_l*enc_l)/(sum_l e_l) via ONE fused bf16 multiply over
    [128, NCH, L, H+1] (e broadcast along h; broadcasts are free) and a
    7-op bf16 binary ADD TREE over l. A ones-column at h=H yields the
    softmax denominator from the same pass.
  * The local walrus build accepts at most ONE semaphore wait per
    instruction; legalize_waits() splits extra waits onto same-engine NoOps.
"""

import numpy as np
import ml_dtypes

import concourse.bass as bass
import concourse.tile as tile
from concourse import mybir
from concourse.masks import make_identity
from concourse.bass_utils import run_bass_kernel_spmd

H = 64
C = 8
L = 96
T = 24
B = 4096
NCORES = 8
BS = B // NCORES          # 512 batch per core
NCH = BS // 128           # 4 partition chunks per core

F32 = mybir.dt.float32
BF16 = mybir.dt.bfloat16
NPBF = ml_dtypes.bfloat16
AF = mybir.ActivationFunctionType
ALU = mybir.AluOpType


def _legalize_waits(nc):
    """This walrus build rejects >1 sem wait per instruction; split extras
    onto same-engine NoOps placed immediately before."""
    cnt = 0
    for bb in nc.main_func.blocks:
        new = []
        for inst in bb.instructions:
            si = inst.sync_info
            if si is not None and len(si.on_wait) > 1:
                waits = list(si.on_wait)
                for w in waits[:-1]:
                    nop = mybir.InstNoOp(name=f"wsplit-{cnt}", ins=[], outs=[])
                    cnt += 1
                    nop.engine = inst.engine
                    nop.sync_info = mybir.SyncInfo(on_wait=[w], on_update=[])
                    new.append(nop)
                inst.sync_info = mybir.SyncInfo(
                    on_wait=[waits[-1]], on_update=list(si.on_update))
            new.append(inst)
        bb.instructions = new
    return cnt


def _tts_raw(nc, eng, out, data0, data1, initial, op0, op1):
    """tensor_tensor_scan without the 2D-shape assert (kept for probes)."""
    return eng.add_instruction(
        mybir.InstTensorScalarPtr(
            name=nc.get_next_instruction_name(),
            is_tensor_tensor_scan=True,
            is_scalar_tensor_tensor=True,
            op0=op0,
            op1=op1,
            ins=[
                eng.lower_ap(data0),
                eng.lower_ap_or_imm(initial),
                eng.lower_ap(data1),
            ],
            outs=[eng.lower_ap(out)],
        )
    )


def _build_program():
    nc = bass.Bass("TRN2", target_bir_lowering=False, debug=False,
                   num_devices=NCORES)

    def din(name, shape, dt=BF16):
        return nc.dram_tensor(name, list(shape), dt, kind="ExternalInput").ap()

    xT = din("xT", (L, C, BS))                  # normalized, transposed, bf16
    enc_w = din("enc_w", (72, 2, 128))          # rows 0:64=Whh_p.T, 64:72=Wih_p.T
    dec_ec = din("dec_ec", (128, 2, 128))       # rows 0:64=wie_p.T, 64:128=wic_p.T
    dec_hh = din("dec_hh", (64, 2, 128))        # dec_Whh_p.T
    w_we = din("w_we", (H, L))                  # attn emb-part We.T
    w_wh = din("w_wh", (H + 1, L))              # attn h-part Wh.T + bias row
    w_emb = din("w_emb", (H, H))                # (emb_W@out_W).T
    w_out = din("w_out", (H, C))                # out_W.T
    b_enc = din("b_enc", (128, 2), F32)         # act biases per pair
    b_dec = din("b_dec", (128, 2), F32)
    emb_bh = din("emb_bh", (H, 1), F32)         # emb_W@out_b + emb_b
    emb0 = din("emb0", (H, 1), F32)             # relu(emb_b)  (t=0 embedding)
    out_b = din("out_b", (C, 1), F32)

    preds = nc.dram_tensor("preds", [T, C, BS], F32, kind="ExternalOutput").ap()

    with tile.TileContext(nc) as tc:
        with (
            tc.tile_pool(name="state", bufs=1) as st,
            tc.tile_pool(name="outp", bufs=2) as outp,
            tc.tile_pool(name="gps", bufs=1, space="PSUM") as gps,
            tc.tile_pool(name="tps", bufs=2, space="PSUM") as tps,
            tc.tile_pool(name="mps", bufs=1, space="PSUM") as mps,
        ):
            # ---------- persistent tiles ----------
            ident_f = st.tile([128, 128], F32)
            make_identity(nc, ident_f[:])
            ident = st.tile([128, 128], BF16)
            nc.scalar.copy(ident[:], ident_f[:])

            w_enc_sb = st.tile([72, 2, 128], BF16)
            w_ec_sb = st.tile([128, 2, 128], BF16)
            w_hh_sb = st.tile([64, 2, 128], BF16)
            w_we_sb = st.tile([H, L], BF16)
            w_wh_sb = st.tile([H + 1, L], BF16)
            w_emb_sb = st.tile([H, H], BF16)
            w_out_sb = st.tile([H, C], BF16)
            b_enc_sb = st.tile([128, 2], F32)
            b_dec_sb = st.tile([128, 2], F32)
            emb_bh_sb = st.tile([H, 1], F32)
            emb0_sb = st.tile([H, 1], F32)
            out_b_sb = st.tile([C, 1], F32)
            for tl, ap in ((w_enc_sb, enc_w), (w_ec_sb, dec_ec),
                           (w_hh_sb, dec_hh), (w_we_sb, w_we),
                           (w_wh_sb, w_wh), (w_emb_sb, w_emb),
                           (w_out_sb, w_out), (b_enc_sb, b_enc),
                           (b_dec_sb, b_dec), (emb_bh_sb, emb_bh),
                           (emb0_sb, emb0), (out_b_sb, out_b)):
                nc.gpsimd.dma_start(tl[:], ap[:])

            # recurrent state: h (bf16) with ones row 64 (attn bias);
            # c lives at partitions 64:128 so the two-input DVE ops have
            # partition-aligned operands (f/o sit at rows 64:128 of the
            # pair tiles); outputs may shift partitions freely.
            h_T = st.tile([H + 1, BS], BF16)
            cb = st.tile([128, BS], BF16)      # c at rows 64:128
            nc.vector.memset(h_T[:], 0.0)
            nc.vector.memset(cb[64:128, :], 0.0)
            nc.vector.memset(h_T[H : H + 1, :], 1.0)

            # encoder outputs [b, chunk, h(65), l] bf16; row H = ones.
            # l innermost: the e-broadcast in the ctx multiply must sit on a
            # NON-inner dim (inner stride-0 drops DVE bf16 from 2 to 1
            # elem/cycle).
            enc_plus = st.tile([128, NCH, H + 1, L], BF16)
            nc.vector.memset(enc_plus[:, :, H, :], 1.0)

            # encoder combined rhs: rows 0:64 = h, 64:72 = x (ping-pong)
            xh = [st.tile([72, BS], BF16, name=f"xh{p}") for p in range(2)]
            for p in range(2):
                nc.vector.memset(xh[p][0:H, :], 0.0)

            # act outputs + cell temps
            if_sb = st.tile([128, BS], BF16)
            go_sb = st.tile([128, BS], BF16)
            t1_sb = st.tile([H, BS], BF16)
            t2_sb = st.tile([H, BS], BF16)
            tc_sb = st.tile([128, BS], BF16)   # used rows 64:128

            # decoder tiles
            ec_sb = st.tile([128, BS], BF16)       # rows 0:64 emb, 64:128 ctx
            e_sb = st.tile([128, NCH, L], BF16)
            rec_sb = st.tile([128, NCH], F32)
            ctx_ch = st.tile([128, NCH, H], BF16)
            P = st.tile([128, NCH, H + 1, L], BF16)
            qa = st.tile([128, NCH, H + 1, 48], BF16)
            qb = st.tile([128, NCH, H + 1, 24], BF16)
            qc = st.tile([128, NCH, H + 1, 12], BF16)
            qd = st.tile([128, NCH, H + 1, 6], BF16)
            qe = st.tile([128, NCH, H + 1, 3], BF16)
            Rt = st.tile([128, NCH, H + 1], BF16)
            St = st.tile([128, NCH, H + 1], BF16)

            # PSUM
            gate_ps = [gps.tile([128, BS], F32, tag=f"gp{p}", name=f"gp{p}")
                       for p in range(2)]

            def lstm_cell(bias_tile):
                """pair PSUMs -> activations -> c/h update (h into dst)."""
                nc.scalar.activation(go_sb[0:H, :], gate_ps[1][0:H, :],
                                     AF.Tanh, bias=bias_tile[0:H, 1:2])
                nc.scalar.activation(if_sb[:], gate_ps[0][:], AF.Sigmoid,
                                     bias=bias_tile[:, 0:1])
                nc.scalar.activation(go_sb[H:128, :], gate_ps[1][H:128, :],
                                     AF.Sigmoid, bias=bias_tile[H:128, 1:2])
                nc.vector.tensor_mul(t1_sb[:], if_sb[0:H, :], go_sb[0:H, :])
                nc.vector.tensor_mul(t2_sb[:], if_sb[H:128, :], cb[H:128, :])
                nc.vector.tensor_add(cb[H:128, :], t1_sb[:], t2_sb[:])
                nc.scalar.activation(tc_sb[H:128, :], cb[H:128, :], AF.Tanh)

            def h_out(dst):
                nc.vector.tensor_mul(dst, go_sb[H:128, :], tc_sb[H:128, :])

            # ------------------ encoder ------------------
            nc.sync.dma_start(xh[0][H:72, :], xT[0])
            for l in range(L):
                if l + 1 < L:
                    nc.sync.dma_start(xh[(l + 1) % 2][H:72, :], xT[l + 1])
                for p in range(2):
                    nc.tensor.matmul(gate_ps[p][:], w_enc_sb[:, p, :],
                                     xh[l % 2][:], start=True, stop=True)
                lstm_cell(b_enc_sb)
                holder = h_T if l == L - 1 else xh[(l + 1) % 2]
                h_out(holder[0:H, :])
                # store h (transposed back to [b, h]) into enc_plus[:,:,l,0:H]
                tp = tps.tile([128, NCH, H], BF16, tag="tp")
                for ci in range(NCH):
                    nc.tensor.transpose(tp[:, ci, :],
                                        holder[0:H, 128 * ci : 128 * (ci + 1)],
                                        ident[0:H, 0:H])
                nc.scalar.copy(enc_plus[:, :, 0:H, l], tp[:])

            # ------------------ decoder ------------------
            for t in range(T):
                # embedding into ec rows 0:64 (bf16)
                if t == 0:
                    nc.vector.tensor_scalar(
                        out=ec_sb[0:H, :],
                        in0=emb0_sb[:, 0:1].broadcast_to((H, BS)),
                        scalar1=0.0, scalar2=None, op0=ALU.add)
                else:
                    emb_ps = mps.tile([H, BS], F32, tag="emb")
                    nc.tensor.matmul(emb_ps[:], w_emb_sb[:], h_T[0:H, :],
                                     start=True, stop=True)
                    nc.scalar.activation(ec_sb[0:H, :], emb_ps[:], AF.Relu,
                                         bias=emb_bh_sb[:, 0:1])

                # attention scores -> e = exp(z)
                zd_ps = mps.tile([128, NCH, L], F32, tag="zd")
                for ci in range(NCH):
                    sl = slice(128 * ci, 128 * (ci + 1))
                    nc.tensor.matmul(zd_ps[:, ci, :], ec_sb[0:H, sl],
                                     w_we_sb[:], start=True, stop=False)
                    nc.tensor.matmul(zd_ps[:, ci, :], h_T[:, sl],
                                     w_wh_sb[:], start=False, stop=True)
                nc.scalar.activation(e_sb[:], zd_ps[:], AF.Exp)

                # ctx: one fused bf16 multiply + bf16 add-tree over l
                e_bc = e_sb[:].unsqueeze(2).broadcast_to((128, NCH, H + 1, L))
                nc.vector.tensor_mul(P[:], enc_plus[:], e_bc)
                nc.vector.tensor_add(qa[:], P[:, :, :, 0:48], P[:, :, :, 48:96])
                nc.vector.tensor_add(qb[:], qa[:, :, :, 0:24], qa[:, :, :, 24:48])
                nc.vector.tensor_add(qc[:], qb[:, :, :, 0:12], qb[:, :, :, 12:24])
                nc.vector.tensor_add(qd[:], qc[:, :, :, 0:6], qc[:, :, :, 6:12])
                nc.vector.tensor_add(qe[:], qd[:, :, :, 0:3], qd[:, :, :, 3:6])
                nc.vector.tensor_add(Rt[:], qe[:, :, :, 0], qe[:, :, :, 1])
                nc.vector.tensor_add(St[:], Rt[:], qe[:, :, :, 2])

                ctxT_ps = mps.tile([H, BS], BF16, tag="ctxT")
                for ci in range(NCH):
                    nc.vector.reciprocal(rec_sb[:, ci : ci + 1],
                                         St[:, ci, H : H + 1])
                    nc.vector.tensor_scalar(
                        out=ctx_ch[:, ci, :], in0=St[:, ci, 0:H],
                        scalar1=rec_sb[:, ci : ci + 1], scalar2=None,
                        op0=ALU.mult)
                    nc.tensor.transpose(ctxT_ps[:, 128 * ci : 128 * (ci + 1)],
                                        ctx_ch[:, ci, :], ident[:])
                nc.scalar.copy(ec_sb[H:128, :], ctxT_ps[:])

                # decoder LSTM cell (comb layer folded into gate weights)
                for p in range(2):
                    nc.tensor.matmul(gate_ps[p][:], w_ec_sb[:, p, :],
                                     ec_sb[:], start=True, stop=False)
                    nc.tensor.matmul(gate_ps[p][:], w_hh_sb[:, p, :],
                                     h_T[0:H, :], start=False, stop=True)
                lstm_cell(b_dec_sb)
                h_out(h_T[0:H, :])

                # prediction -> output store
                pred_ps = mps.tile([C, BS], F32, tag="pred")
                nc.tensor.matmul(pred_ps[:], w_out_sb[:], h_T[0:H, :],
                                 start=True, stop=True)
                po = outp.tile([C, BS], F32, tag="po")
                nc.scalar.activation(po[:], pred_ps[:], AF.Identity,
                                     bias=out_b_sb[:, 0:1])
                nc.sync.dma_start(preds[t], po[:])

    _legalize_waits(nc)
    return nc


_NC_CACHE = []


def _get_nc():
    if not _NC_CACHE:
        _NC_CACHE.append(_build_program())
    return _NC_CACHE[0]


def _bf(x):
    return np.ascontiguousarray(np.asarray(x, np.float32).astype(NPBF))


def _prep_weights(i):
    """Host-side packing. Gate pairs: p0=(i,f), p1=(g,o) in pytorch row order."""
    Wih = np.asarray(i["enc_Wih"], np.float32)
    Whh = np.asarray(i["enc_Whh"], np.float32)
    be = np.asarray(i["enc_bih"] + i["enc_bhh"], np.float32)

    enc_w = np.zeros((72, 2, 128), np.float32)
    for p in range(2):
        r = slice(128 * p, 128 * (p + 1))
        enc_w[0:64, p, :] = Whh[r].T
        enc_w[64:72, p, :] = Wih[r].T

    emb_W = np.asarray(i["emb_W"], np.float32)
    emb_b = np.asarray(i["emb_b"], np.float32)
    attn_W = np.asarray(i["attn_W"], np.float32)
    attn_b = np.asarray(i["attn_b"], np.float32)
    comb_W = np.asarray(i["comb_W"], np.float32)
    comb_b = np.asarray(i["comb_b"], np.float32)
    dWih = np.asarray(i["dec_Wih"], np.float32)
    dWhh = np.asarray(i["dec_Whh"], np.float32)
    bd = (np.asarray(i["dec_bih"] + i["dec_bhh"], np.float32)
          + dWih @ comb_b)
    out_W = np.asarray(i["out_W"], np.float32)
    out_bv = np.asarray(i["out_b"], np.float32)

    wie = dWih @ comb_W[:, :H]
    wic = dWih @ comb_W[:, H:]
    dec_ec = np.zeros((128, 2, 128), np.float32)
    dec_hh = np.zeros((64, 2, 128), np.float32)
    for p in range(2):
        r = slice(128 * p, 128 * (p + 1))
        dec_ec[0:64, p, :] = wie[r].T
        dec_ec[64:128, p, :] = wic[r].T
        dec_hh[:, p, :] = dWhh[r].T

    w_wh = np.zeros((H + 1, L), np.float32)
    w_wh[0:H] = attn_W[:, H:].T
    w_wh[H] = attn_b

    def bias_pack(b):
        out = np.zeros((128, 2), np.float32)
        out[:, 0] = b[0:128]
        out[0:64, 1] = b[128:192]
        out[64:128, 1] = b[192:256]
        return out

    return dict(
        enc_w=_bf(enc_w), dec_ec=_bf(dec_ec), dec_hh=_bf(dec_hh),
        w_we=_bf(attn_W[:, :H].T), w_wh=_bf(w_wh),
        w_emb=_bf((emb_W @ out_W).T), w_out=_bf(out_W.T),
        b_enc=bias_pack(be), b_dec=bias_pack(bd),
        emb_bh=(emb_W @ out_bv + emb_b).reshape(H, 1).astype(np.float32),
        emb0=np.maximum(emb_b, 0.0).reshape(H, 1).astype(np.float32),
        out_b=out_bv.reshape(C, 1).astype(np.float32),
    )


def kernel(**inputs):
    x_enc = np.asarray(inputs["x_enc"], np.float32)
    seq_last = x_enc[:, -1:, :]                       # [B, 1, C]
    x = x_enc - seq_last                              # [B, L, C]

    weights = _prep_weights({k: np.asarray(v) for k, v in inputs.items()
                             if k not in ("x_enc", "x_mark_enc", "x_dec",
                                          "x_mark_dec")})

    core_ids = list(range(NCORES))
    in_maps = []
    for ci in core_ids:
        xs = x[ci * BS : (ci + 1) * BS]               # [BS, L, C]
        xTc = np.ascontiguousarray(
            xs.transpose(1, 2, 0).astype(NPBF))       # [L, C, BS] bf16
        m = dict(weights)
        m["xT"] = xTc
        in_maps.append(m)

    nc = _get_nc()
    res = run_bass_kernel_spmd(nc, in_maps, core_ids)
    global LAST_RESULTS
    LAST_RESULTS = res

    out = np.empty((B, T, C), np.float32)
    for ci in core_ids:
        p = res.results[ci]["preds"]                  # [T, C, BS]
        out[ci * BS : (ci + 1) * BS] = p.transpose(2, 0, 1)
    out += seq_last
    return out


# revision 21
# speedup vs baseline: 2.2617x; 1.0169x over previous
"""Attn_LSTM Trainium2 kernel — 8-core data-parallel Bass/Tile implementation.

Model (per reference): 1-layer LSTM encoder over L=96 steps, then T=24
attention-decoder steps. B=4096 sharded 512/core across 8 NeuronCores;
weights replicated.

Device-side design (driven by measured engine rates):
  * PE matmuls all-bf16 (fp32 PSUM accumulation), gates paired (i,f)/(g,o)
    into [128,512] matmuls with K-stacked inputs ([h;x] K=72 encoder,
    [emb;ctx] K=128 + h K=64 decoder).
  * DVE: only TensorTensor (1 elem/cyc/lane fp32, 2/cyc pure-bf16) and
    TensorScalar (2/cyc) ops — scalar_tensor_tensor and tensor_tensor_scan
    are microcoded ~8-20x slower on this DVE and are avoided. GpSimd (Pool)
    is erratic/slow and unused for compute.
  * Cell: sigmoid/tanh activations on the ACT engine (cost ~0.84ns/col,
    independent of partition count, so partition-paired gates are free);
    cell math is 4 pure-bf16 tensor_tensor ops. States bf16.
  * Attention context: softmax numerators e=exp(z) from one ACT call; then
    ctx = (sum_l e_l*enc_l)/(sum_l e_l) via ONE fused bf16 multiply over
    [128, NCH, L, H+1] (e broadcast along h; broadcasts are free) and a
    7-op bf16 binary ADD TREE over l. A ones-column at h=H yields the
    softmax denominator from the same pass.
  * The local walrus build accepts at most ONE semaphore wait per
    instruction; legalize_waits() splits extra waits onto same-engine NoOps.
"""

import numpy as np
import ml_dtypes

import concourse.bass as bass
import concourse.tile as tile
from concourse import mybir
from concourse.masks import make_identity
from concourse.bass_utils import run_bass_kernel_spmd

H = 64
C = 8
L = 96
T = 24
B = 4096
NCORES = 8
BS = B // NCORES          # 512 batch per core
NCH = BS // 128           # 4 partition chunks per core

F32 = mybir.dt.float32
BF16 = mybir.dt.bfloat16
NPBF = ml_dtypes.bfloat16
AF = mybir.ActivationFunctionType
ALU = mybir.AluOpType


def _legalize_waits(nc):
    """This walrus build rejects >1 sem wait per instruction; split extras
    onto same-engine NoOps placed immediately before."""
    cnt = 0
    for bb in nc.main_func.blocks:
        new = []
        for inst in bb.instructions:
            si = inst.sync_info
            if si is not None and len(si.on_wait) > 1:
                waits = list(si.on_wait)
                for w in waits[:-1]:
                    nop = mybir.InstNoOp(name=f"wsplit-{cnt}", ins=[], outs=[])
                    cnt += 1
                    nop.engine = inst.engine
                    nop.sync_info = mybir.SyncInfo(on_wait=[w], on_update=[])
                    new.append(nop)
                inst.sync_info = mybir.SyncInfo(
                    on_wait=[waits[-1]], on_update=list(si.on_update))
            new.append(inst)
        bb.instructions = new
    return cnt


def _tts_raw(nc, eng, out, data0, data1, initial, op0, op1):
    """tensor_tensor_scan without the 2D-shape assert (kept for probes)."""
    return eng.add_instruction(
        mybir.InstTensorScalarPtr(
            name=nc.get_next_instruction_name(),
            is_tensor_tensor_scan=True,
            is_scalar_tensor_tensor=True,
            op0=op0,
            op1=op1,
            ins=[
                eng.lower_ap(data0),
                eng.lower_ap_or_imm(initial),
                eng.lower_ap(data1),
            ],
            outs=[eng.lower_ap(out)],
        )
    )


def _build_program():
    nc = bass.Bass("TRN2", target_bir_lowering=False, debug=False,
                   num_devices=NCORES)

    def din(name, shape, dt=BF16):
        return nc.dram_tensor(name, list(shape), dt, kind="ExternalInput").ap()

    xT = din("xT", (L, C, BS))                  # normalized, transposed, bf16
    enc_w = din("enc_w", (72, 2, 128))          # rows 0:64=Whh_p.T, 64:72=Wih_p.T
    dec_ec = din("dec_ec", (128, 2, 128))       # rows 0:64=wie_p.T, 64:128=wic_p.T
    dec_hh = din("dec_hh", (64, 2, 128))        # dec_Whh_p.T
    w_we = din("w_we", (H, L))                  # attn emb-part We.T
    w_wh = din("w_wh", (H + 1, L))              # attn h-part Wh.T + bias row
    w_emb = din("w_emb", (H, H))                # (emb_W@out_W).T
    w_out = din("w_out", (H, C))                # out_W.T
    b_enc = din("b_enc", (128, 2), F32)         # act biases per pair
    b_dec = din("b_dec", (128, 2), F32)
    emb_bh = din("emb_bh", (H, 1), F32)         # emb_W@out_b + emb_b
    emb0 = din("emb0", (H, 1), F32)             # relu(emb_b)  (t=0 embedding)
    out_b = din("out_b", (C, 1), F32)

    preds = nc.dram_tensor("preds", [T, C, BS], F32, kind="ExternalOutput").ap()

    with tile.TileContext(nc) as tc:
        with (
            tc.tile_pool(name="state", bufs=1) as st,
            tc.tile_pool(name="outp", bufs=2) as outp,
            tc.tile_pool(name="gps", bufs=1, space="PSUM") as gps,
            tc.tile_pool(name="tps", bufs=1, space="PSUM") as tps,
            tc.tile_pool(name="mps", bufs=1, space="PSUM") as mps,
        ):
            # ---------- persistent tiles ----------
            ident_f = st.tile([128, 128], F32)
            make_identity(nc, ident_f[:])
            ident = st.tile([128, 128], BF16)
            nc.scalar.copy(ident[:], ident_f[:])

            w_enc_sb = st.tile([72, 2, 128], BF16)
            w_ec_sb = st.tile([128, 2, 128], BF16)
            w_hh_sb = st.tile([64, 2, 128], BF16)
            w_we_sb = st.tile([H, L], BF16)
            w_wh_sb = st.tile([H + 1, L], BF16)
            w_emb_sb = st.tile([H, H], BF16)
            w_out_sb = st.tile([H, C], BF16)
            b_enc_sb = st.tile([128, 2], F32)
            b_dec_sb = st.tile([128, 2], F32)
            emb_bh_sb = st.tile([H, 1], F32)
            emb0_sb = st.tile([H, 1], F32)
            out_b_sb = st.tile([C, 1], F32)
            for tl, ap in ((w_enc_sb, enc_w), (w_ec_sb, dec_ec),
                           (w_hh_sb, dec_hh), (w_we_sb, w_we),
                           (w_wh_sb, w_wh), (w_emb_sb, w_emb),
                           (w_out_sb, w_out), (b_enc_sb, b_enc),
                           (b_dec_sb, b_dec), (emb_bh_sb, emb_bh),
                           (emb0_sb, emb0), (out_b_sb, out_b)):
                nc.gpsimd.dma_start(tl[:], ap[:])

            # recurrent state: h (bf16) with ones row 64 (attn bias);
            # c lives at partitions 64:128 so the two-input DVE ops have
            # partition-aligned operands (f/o sit at rows 64:128 of the
            # pair tiles); outputs may shift partitions freely.
            h_T = st.tile([H + 1, BS], BF16)
            cb = st.tile([128, BS], BF16)      # c at rows 64:128
            nc.vector.memset(h_T[:], 0.0)
            nc.vector.memset(cb[64:128, :], 0.0)
            nc.vector.memset(h_T[H : H + 1, :], 1.0)

            # encoder outputs [b, chunk, h(65), l] bf16; row H = ones.
            # l innermost: the e-broadcast in the ctx multiply must sit on a
            # NON-inner dim (inner stride-0 drops DVE bf16 from 2 to 1
            # elem/cycle).
            enc_plus = st.tile([128, NCH, H + 1, L], BF16)
            nc.vector.memset(enc_plus[:, :, H, :], 1.0)

            # encoder combined rhs: rows 0:64 = h, 64:72 = x (ping-pong)
            xh = [st.tile([72, BS], BF16, name=f"xh{p}") for p in range(2)]
            for p in range(2):
                nc.vector.memset(xh[p][0:H, :], 0.0)

            # act outputs + cell temps
            if_sb = st.tile([128, BS], BF16)
            go_sb = st.tile([128, BS], BF16)
            t1_sb = st.tile([H, BS], BF16)
            t2_sb = st.tile([H, BS], BF16)
            tc_sb = st.tile([128, BS], BF16)   # used rows 64:128

            # decoder tiles
            ec_sb = st.tile([128, BS], BF16)       # rows 0:64 emb, 64:128 ctx
            e_sb = st.tile([128, NCH, L], BF16)
            rec_sb = st.tile([128, NCH], F32)
            ctx_ch = st.tile([128, NCH, H], BF16)
            P = st.tile([128, NCH, H + 1, L], BF16)
            qa = st.tile([128, NCH, H + 1, 48], BF16)
            qb = st.tile([128, NCH, H + 1, 24], BF16)
            qc = st.tile([128, NCH, H + 1, 12], BF16)
            qd = st.tile([128, NCH, H + 1, 6], BF16)
            qe = st.tile([128, NCH, H + 1, 3], BF16)
            Rt = st.tile([128, NCH, H + 1], BF16)
            St = st.tile([128, NCH, H + 1], BF16)

            # PSUM
            gate_ps = [gps.tile([128, BS], F32, tag=f"gp{p}", name=f"gp{p}")
                       for p in range(2)]

            def lstm_cell(bias_tile):
                """pair PSUMs -> activations -> c/h update (h into dst)."""
                nc.scalar.activation(go_sb[0:H, :], gate_ps[1][0:H, :],
                                     AF.Tanh, bias=bias_tile[0:H, 1:2])
                nc.scalar.activation(if_sb[:], gate_ps[0][:], AF.Sigmoid,
                                     bias=bias_tile[:, 0:1])
                nc.scalar.activation(go_sb[H:128, :], gate_ps[1][H:128, :],
                                     AF.Sigmoid, bias=bias_tile[H:128, 1:2])
                nc.vector.tensor_mul(t1_sb[:], if_sb[0:H, :], go_sb[0:H, :])
                nc.vector.tensor_mul(t2_sb[:], if_sb[H:128, :], cb[H:128, :])
                nc.vector.tensor_add(cb[H:128, :], t1_sb[:], t2_sb[:])
                nc.scalar.activation(tc_sb[H:128, :], cb[H:128, :], AF.Tanh)

            def h_out(dst):
                nc.vector.tensor_mul(dst, go_sb[H:128, :], tc_sb[H:128, :])

            # ------------------ encoder ------------------
            nc.sync.dma_start(xh[0][H:72, :], xT[0])
            for l in range(L):
                if l + 1 < L:
                    nc.sync.dma_start(xh[(l + 1) % 2][H:72, :], xT[l + 1])
                for p in range(2):
                    nc.tensor.matmul(gate_ps[p][:], w_enc_sb[:, p, :],
                                     xh[l % 2][:], start=True, stop=True)
                lstm_cell(b_enc_sb)
                holder = h_T if l == L - 1 else xh[(l + 1) % 2]
                h_out(holder[0:H, :])
                # store h (transposed back to [b, h]) into enc_plus[:,:,0:H,l].
                # Transpose via a REAL matmul against identity so the PSUM
                # result is fp32 (4-byte aligned at any l offset); 4 steps
                # land strided in one PSUM tile and one copy per 4 steps
                # amortizes the strided enc write.
                if l % 4 == 0:
                    tp4 = tps.tile([128, NCH, H, 4], F32, tag="tp4")
                for ci in range(NCH):
                    nc.tensor.matmul(tp4[:, ci, :, l % 4],
                                     holder[0:H, 128 * ci : 128 * (ci + 1)],
                                     ident[0:H, 0:H], start=True, stop=True)
                if l % 4 == 3:
                    nc.scalar.copy(enc_plus[:, :, 0:H, l - 3 : l + 1], tp4[:])

            # ------------------ decoder ------------------
            for t in range(T):
                # embedding into ec rows 0:64 (bf16)
                if t == 0:
                    nc.vector.tensor_scalar(
                        out=ec_sb[0:H, :],
                        in0=emb0_sb[:, 0:1].broadcast_to((H, BS)),
                        scalar1=0.0, scalar2=None, op0=ALU.add)
                else:
                    emb_ps = mps.tile([H, BS], F32, tag="emb")
                    nc.tensor.matmul(emb_ps[:], w_emb_sb[:], h_T[0:H, :],
                                     start=True, stop=True)
                    nc.scalar.activation(ec_sb[0:H, :], emb_ps[:], AF.Relu,
                                         bias=emb_bh_sb[:, 0:1])

                # attention scores -> e = exp(z)
                zd_ps = mps.tile([128, NCH, L], F32, tag="zd")
                for ci in range(NCH):
                    sl = slice(128 * ci, 128 * (ci + 1))
                    nc.tensor.matmul(zd_ps[:, ci, :], ec_sb[0:H, sl],
                                     w_we_sb[:], start=True, stop=False)
                    nc.tensor.matmul(zd_ps[:, ci, :], h_T[:, sl],
                                     w_wh_sb[:], start=False, stop=True)
                nc.scalar.activation(e_sb[:], zd_ps[:], AF.Exp)

                # ctx: one fused bf16 multiply + bf16 add-tree over l
                e_bc = e_sb[:].unsqueeze(2).broadcast_to((128, NCH, H + 1, L))
                nc.vector.tensor_mul(P[:], enc_plus[:], e_bc)
                nc.vector.tensor_add(qa[:], P[:, :, :, 0:48], P[:, :, :, 48:96])
                nc.vector.tensor_add(qb[:], qa[:, :, :, 0:24], qa[:, :, :, 24:48])
                nc.vector.tensor_add(qc[:], qb[:, :, :, 0:12], qb[:, :, :, 12:24])
                nc.vector.tensor_add(qd[:], qc[:, :, :, 0:6], qc[:, :, :, 6:12])
                nc.vector.tensor_add(qe[:], qd[:, :, :, 0:3], qd[:, :, :, 3:6])
                nc.vector.tensor_add(Rt[:], qe[:, :, :, 0], qe[:, :, :, 1])
                nc.vector.tensor_add(St[:], Rt[:], qe[:, :, :, 2])

                ctxT_ps = mps.tile([H, BS], BF16, tag="ctxT")
                for ci in range(NCH):
                    nc.vector.reciprocal(rec_sb[:, ci : ci + 1],
                                         St[:, ci, H : H + 1])
                    nc.vector.tensor_scalar(
                        out=ctx_ch[:, ci, :], in0=St[:, ci, 0:H],
                        scalar1=rec_sb[:, ci : ci + 1], scalar2=None,
                        op0=ALU.mult)
                    nc.tensor.transpose(ctxT_ps[:, 128 * ci : 128 * (ci + 1)],
                                        ctx_ch[:, ci, :], ident[:])
                nc.scalar.copy(ec_sb[H:128, :], ctxT_ps[:])

                # decoder LSTM cell (comb layer folded into gate weights)
                for p in range(2):
                    nc.tensor.matmul(gate_ps[p][:], w_ec_sb[:, p, :],
                                     ec_sb[:], start=True, stop=False)
                    nc.tensor.matmul(gate_ps[p][:], w_hh_sb[:, p, :],
                                     h_T[0:H, :], start=False, stop=True)
                lstm_cell(b_dec_sb)
                h_out(h_T[0:H, :])

                # prediction -> output store
                pred_ps = mps.tile([C, BS], F32, tag="pred")
                nc.tensor.matmul(pred_ps[:], w_out_sb[:], h_T[0:H, :],
                                 start=True, stop=True)
                po = outp.tile([C, BS], F32, tag="po")
                nc.scalar.activation(po[:], pred_ps[:], AF.Identity,
                                     bias=out_b_sb[:, 0:1])
                nc.sync.dma_start(preds[t], po[:])

    _legalize_waits(nc)
    return nc


_NC_CACHE = []


def _get_nc():
    if not _NC_CACHE:
        _NC_CACHE.append(_build_program())
    return _NC_CACHE[0]


def _bf(x):
    return np.ascontiguousarray(np.asarray(x, np.float32).astype(NPBF))


def _prep_weights(i):
    """Host-side packing. Gate pairs: p0=(i,f), p1=(g,o) in pytorch row order."""
    Wih = np.asarray(i["enc_Wih"], np.float32)
    Whh = np.asarray(i["enc_Whh"], np.float32)
    be = np.asarray(i["enc_bih"] + i["enc_bhh"], np.float32)

    enc_w = np.zeros((72, 2, 128), np.float32)
    for p in range(2):
        r = slice(128 * p, 128 * (p + 1))
        enc_w[0:64, p, :] = Whh[r].T
        enc_w[64:72, p, :] = Wih[r].T

    emb_W = np.asarray(i["emb_W"], np.float32)
    emb_b = np.asarray(i["emb_b"], np.float32)
    attn_W = np.asarray(i["attn_W"], np.float32)
    attn_b = np.asarray(i["attn_b"], np.float32)
    comb_W = np.asarray(i["comb_W"], np.float32)
    comb_b = np.asarray(i["comb_b"], np.float32)
    dWih = np.asarray(i["dec_Wih"], np.float32)
    dWhh = np.asarray(i["dec_Whh"], np.float32)
    bd = (np.asarray(i["dec_bih"] + i["dec_bhh"], np.float32)
          + dWih @ comb_b)
    out_W = np.asarray(i["out_W"], np.float32)
    out_bv = np.asarray(i["out_b"], np.float32)

    wie = dWih @ comb_W[:, :H]
    wic = dWih @ comb_W[:, H:]
    dec_ec = np.zeros((128, 2, 128), np.float32)
    dec_hh = np.zeros((64, 2, 128), np.float32)
    for p in range(2):
        r = slice(128 * p, 128 * (p + 1))
        dec_ec[0:64, p, :] = wie[r].T
        dec_ec[64:128, p, :] = wic[r].T
        dec_hh[:, p, :] = dWhh[r].T

    w_wh = np.zeros((H + 1, L), np.float32)
    w_wh[0:H] = attn_W[:, H:].T
    w_wh[H] = attn_b

    def bias_pack(b):
        out = np.zeros((128, 2), np.float32)
        out[:, 0] = b[0:128]
        out[0:64, 1] = b[128:192]
        out[64:128, 1] = b[192:256]
        return out

    return dict(
        enc_w=_bf(enc_w), dec_ec=_bf(dec_ec), dec_hh=_bf(dec_hh),
        w_we=_bf(attn_W[:, :H].T), w_wh=_bf(w_wh),
        w_emb=_bf((emb_W @ out_W).T), w_out=_bf(out_W.T),
        b_enc=bias_pack(be), b_dec=bias_pack(bd),
        emb_bh=(emb_W @ out_bv + emb_b).reshape(H, 1).astype(np.float32),
        emb0=np.maximum(emb_b, 0.0).reshape(H, 1).astype(np.float32),
        out_b=out_bv.reshape(C, 1).astype(np.float32),
    )


def kernel(**inputs):
    x_enc = np.asarray(inputs["x_enc"], np.float32)
    seq_last = x_enc[:, -1:, :]                       # [B, 1, C]
    x = x_enc - seq_last                              # [B, L, C]

    weights = _prep_weights({k: np.asarray(v) for k, v in inputs.items()
                             if k not in ("x_enc", "x_mark_enc", "x_dec",
                                          "x_mark_dec")})

    core_ids = list(range(NCORES))
    in_maps = []
    for ci in core_ids:
        xs = x[ci * BS : (ci + 1) * BS]               # [BS, L, C]
        xTc = np.ascontiguousarray(
            xs.transpose(1, 2, 0).astype(NPBF))       # [L, C, BS] bf16
        m = dict(weights)
        m["xT"] = xTc
        in_maps.append(m)

    nc = _get_nc()
    res = run_bass_kernel_spmd(nc, in_maps, core_ids)
    global LAST_RESULTS
    LAST_RESULTS = res

    out = np.empty((B, T, C), np.float32)
    for ci in core_ids:
        p = res.results[ci]["preds"]                  # [T, C, BS]
        out[ci * BS : (ci + 1) * BS] = p.transpose(2, 0, 1)
    out += seq_last
    return out
